# revision 4
# baseline (speedup 1.0000x reference)
"""BlockDecay (RetNet-style chunkwise linear attention with per-feature decay)
Trainium2 Bass kernel, batch-parallel over 8 NeuronCores.

Math (per batch): out[t] = sum_r q[t,r] * S_t[r,:],
  S_t[r,d] = sum_{s<=t} gamma_r^{t-s} k[s,r] h[s,d]
computed chunkwise with C=256 using the standard factorization
  A[i,j] = (q gamma^i) . (k gamma^-j),  intra = (A*mask) @ h,
  inter  = (q gamma^{i+1}) @ S_chunk,   S' = gamma^{256} S' + K',
  K'[r,d] = sum_j gamma_r^{256-j} k[j,r] h[j,d]   (S' = gamma*S folded scale)

All scaling/transposition is done on the host; the device sees:
  qsT [R, W] = (q * gamma^(i%C)).T          (f32r)
  ksT [R, W] = (k * gamma^-(j%C)).T         (f32r)
  k2n [128, W]  block-local [j, (blk, r)] = k*gamma^(C - j%C)   (f32r)
  hn  [128, W]  block-local [j, (blk, d)]                        (f32r)
  mkT [128, 512] = [mask0T | mask1T] for the 256-wide causal mask
  g256 [128, 1] = gamma^256
Output otT [D, W] (transposed), host transposes back.
"""
import os
import sys
import numpy as np

for _p in ("/root/.axon_site", "/root/.axon_site/_ro/trn_rl_repo",
           "/root/.axon_site/_ro/pypackages"):
    if _p not in sys.path and os.path.isdir(_p):
        sys.path.append(_p)

B, W, R, D = 8, 4096, 128, 128
C = 256
NCH = W // C
NBLK = W // 128

_PROG = {}


def _patched_tc(nc):
    """TileContext whose exit drain splits sem waits across multiple drains
    (this walrus build only accepts one sync-wait per ctrl instruction)."""
    import concourse.tile as tile
    import concourse.tile_sem_assignment as tsa
    from concourse.tile import ScopedClock

    class PatchedTileContext(tile.TileContext):
        def _drain_and_barrier(self, tick_clock, wait_clock):
            gc = tick_clock.global_clock
            n = tsa.N_PROCS
            for p in range(n):
                ticks = gc[p]
                if ticks <= 0:
                    continue
                d = self.nc.sync.drain()
                wait_clock.add_sem_waits(
                    d.ins,
                    ScopedClock({None: tsa.VectorClock(
                        [ticks if q == p else 0 for q in range(n)])}),
                )
            self.nc.all_engine_barrier()
            assert self.sems is not None
            popped = self.nc._tile_sem_poison_stack.pop()
            assert popped is self._sem_poison
            self.nc.clear_and_free_semaphores(list(self.sems.allocated().values()))
            self.nc.all_engine_barrier()

    return PatchedTileContext(nc)


def _split_multi_waits(nc, limit=1):
    """This container's walrus accepts only one sync-wait per instruction:
    hoist extra waits onto injected same-engine NoOps (engines are in-order,
    so waiting earlier in the stream is always safe)."""
    import concourse.mybir as mybir
    n_new = 0
    for fn in nc.m.functions:
        for bb in fn.blocks:
            out = []
            changed = False
            for inst in bb.instructions:
                si = getattr(inst, "sync_info", None)
                waits = list(si.on_wait) if si is not None and si.on_wait else []
                if len(waits) > limit:
                    for w in waits[:-limit]:
                        nop = mybir.InstNoOp(
                            name=f"I-wsplit-{n_new}",
                            engine=inst.engine,
                            sync_info=mybir.SyncInfo(on_wait=[w], on_update=[]),
                        )
                        n_new += 1
                        out.append(nop)
                    si.on_wait = waits[-limit:]
                    changed = True
                out.append(inst)
            if changed:
                bb.instructions = out
    return n_new


def _build_program(use_f32r=True):
    key = ("v1", use_f32r)
    if key in _PROG:
        return _PROG[key]
    import concourse.bass as bass
    import concourse.mybir as mybir

    F32 = mybir.dt.float32
    FM = mybir.dt.float32r if use_f32r else F32  # matmul operand dtype

    nc = bass.Bass()
    qsT = nc.declare_dram_parameter("qsT", [128, W], FM, isOutput=False)
    ksT = nc.declare_dram_parameter("ksT", [128, W], FM, isOutput=False)
    k2n = nc.declare_dram_parameter("k2n", [128, W], FM, isOutput=False)
    hn = nc.declare_dram_parameter("hn", [128, W], FM, isOutput=False)
    mkT = nc.declare_dram_parameter("mkT", [128, 512], F32, isOutput=False)
    g256 = nc.declare_dram_parameter("g256", [128, 1], F32, isOutput=False)
    otT = nc.declare_dram_parameter("otT", [128, W], F32, isOutput=True)

    mm = nc.tensor.matmul
    with _patched_tc(nc) as tc:
        with tc.tile_pool(name="big", bufs=1) as big, \
             tc.tile_pool(name="small", bufs=1) as small, \
             tc.tile_pool(name="st", bufs=4) as stp, \
             tc.tile_pool(name="am", bufs=3) as amp, \
             tc.tile_pool(name="ps_at", bufs=2, space="PSUM") as ps_at, \
             tc.tile_pool(name="ps_ot", bufs=2, space="PSUM") as ps_ot, \
             tc.tile_pool(name="ps_kp", bufs=2, space="PSUM") as ps_kp:

            qsT_sb = big.tile([128, W], FM, tag="qsT")
            ksT_sb = big.tile([128, W], FM, tag="ksT")
            k2n_sb = big.tile([128, W], FM, tag="k2n")
            hn_sb = big.tile([128, W], FM, tag="hn")
            otT_sb = big.tile([128, W], F32, tag="otT")
            mkT_sb = small.tile([128, 512], F32, tag="mkT")
            g256_sb = small.tile([128, 1], F32, tag="g256")

            nc.sync.dma_start(mkT_sb[:], mkT[:])
            nc.sync.dma_start(g256_sb[:], g256[:])
            # inputs in 4 pieces each, interleaved so early chunks land first
            P = W // 4
            for p in range(4):
                s = slice(p * P, (p + 1) * P)
                nc.sync.dma_start(qsT_sb[:, s], qsT[:, s])
                nc.sync.dma_start(ksT_sb[:, s], ksT[:, s])
                nc.sync.dma_start(k2n_sb[:, s], k2n[:, s])
                nc.sync.dma_start(hn_sb[:, s], hn[:, s])

            S_prev = stp.tile([128, 128], FM, tag="S")
            if use_f32r:
                z = small.tile([128, 128], F32, tag="z")
                nc.vector.memset(z[:], 0.0)
                nc.vector.tensor_copy(S_prev[:], z[:])
            else:
                nc.vector.memset(S_prev[:], 0.0)

            # software-pipelined: emit chunk m's independent work, then the
            # output-forming group of chunk m-1
            pend = None  # (m, S_of_chunk, Am0, Am1)
            for m in range(NCH):
                c = m * C
                j0 = slice(c, c + 128)
                j1 = slice(c + 128, c + 256)
                ci = slice(c, c + 256)

                KP = ps_kp.tile([128, 128], mybir.dt.float32, tag="kp")
                mm(KP[:], k2n_sb[:, j0], hn_sb[:, j0], start=True, stop=False)
                mm(KP[:], k2n_sb[:, j1], hn_sb[:, j1], start=False, stop=True)
                S_new = stp.tile([128, 128], FM, tag="S")
                nc.vector.scalar_tensor_tensor(
                    out=S_new[:], in0=S_prev[:], scalar=g256_sb[:, 0:1],
                    in1=KP[:], op0=mybir.AluOpType.mult, op1=mybir.AluOpType.add)

                AT0 = ps_at.tile([128, 256], mybir.dt.float32, tag="at0")
                AT1 = ps_at.tile([128, 256], mybir.dt.float32, tag="at1")
                mm(AT0[:], ksT_sb[:, j0], qsT_sb[:, ci], start=True, stop=True)
                mm(AT1[:], ksT_sb[:, j1], qsT_sb[:, ci], start=True, stop=True)
                Am0 = amp.tile([128, 256], FM, tag="am0")
                Am1 = amp.tile([128, 256], FM, tag="am1")
                nc.vector.tensor_mul(Am0[:], AT0[:], mkT_sb[:, 0:256])
                nc.vector.tensor_mul(Am1[:], AT1[:], mkT_sb[:, 256:512])

                if pend is not None:
                    _emit_out(nc, mm, pend, hn_sb, qsT_sb, otT_sb, otT, ps_ot)
                pend = (m, S_prev, Am0, Am1)
                S_prev = S_new
            _emit_out(nc, mm, pend, hn_sb, qsT_sb, otT_sb, otT, ps_ot)

    _split_multi_waits(nc)
    _PROG[key] = nc
    return nc


def _emit_out(nc, mm, pend, hn_sb, qsT_sb, otT_sb, otT, ps_ot):
    import concourse.mybir as mybir
    m, S_m, Am0, Am1 = pend
    c = m * C
    j0 = slice(c, c + 128)
    j1 = slice(c + 128, c + 256)
    ci = slice(c, c + 256)
    OT = ps_ot.tile([128, 256], mybir.dt.float32, tag="ot")
    mm(OT[:], hn_sb[:, j0], Am0[:], start=True, stop=False)
    mm(OT[:], hn_sb[:, j1], Am1[:], start=False, stop=False)
    mm(OT[:], S_m[:], qsT_sb[:, ci], start=False, stop=True)
    nc.scalar.copy(otT_sb[:, ci], OT[:])
    if m % 4 == 3:
        s = slice((m - 3) * C, (m + 1) * C)
        nc.sync.dma_start(otT[:, s], otT_sb[:, s])


def _host_prep(q_alpha, k, h_norm, gamma_vec, causal_mask):
    """Build per-core input maps (all float32, C-contiguous)."""
    gamma = np.clip(gamma_vec.astype(np.float64), 1e-8, None)
    log_g = np.log(gamma)
    i_loc = (np.arange(W) % C).astype(np.float64)
    Sq = np.exp(np.outer(i_loc, log_g))          # [W, R] gamma^(i%C)
    Skneg = np.exp(np.outer(-i_loc, log_g))      # gamma^-(j%C)
    Sk2 = np.exp(np.outer(C - i_loc, log_g))     # gamma^(C - j%C)
    g256 = np.exp(C * log_g).astype(np.float32).reshape(128, 1)

    M256 = np.zeros((C, C), np.float32)
    M256[:128, :128] = causal_mask
    M256[128:, :128] = 1.0
    M256[128:, 128:] = causal_mask
    mkT = np.concatenate([M256[:, :128].T, M256[:, 128:].T], axis=1)  # [128,512]
    mkT = np.ascontiguousarray(mkT, np.float32)

    def blockify(x):  # [W, 128] -> [128, (blk, 128)]
        return np.ascontiguousarray(
            x.reshape(NBLK, 128, 128).transpose(1, 0, 2).reshape(128, W))

    in_maps = []
    for b in range(B):
        q64 = q_alpha[b].astype(np.float64)
        k64 = k[b].astype(np.float64)
        in_maps.append({
            "qsT": np.ascontiguousarray((q64 * Sq).T.astype(np.float32)),
            "ksT": np.ascontiguousarray((k64 * Skneg).T.astype(np.float32)),
            "k2n": blockify((k64 * Sk2).astype(np.float32)),
            "hn": blockify(np.ascontiguousarray(h_norm[b], np.float32)),
            "mkT": mkT,
            "g256": g256,
        })
    return in_maps


def _ensure_ntff_hook():
    """Register the axon NTFF profile hook if the container's antenv lacks it."""
    try:
        from antenv import axon_hooks  # noqa: F401
        return
    except ImportError:
        pass
    import types
    import antenv
    try:
        import trn_agent_boot.trn_boot as tb
        hook = tb._ntff_profile_via_ctypes("/opt/axon/libaxon_pjrt.so")
    except Exception:
        hook = None
    mod = types.ModuleType("antenv.axon_hooks")
    mod.get_axon_ntff_profile_hook = lambda: hook
    mod.set_axon_ntff_profile_hook = lambda h: None
    sys.modules["antenv.axon_hooks"] = mod
    antenv.axon_hooks = mod


_last = {"exec_time_ns": None}


def kernel(q_alpha, k, h_norm, gamma_vec, causal_mask, decay_diff,
           _trace=False, _use_f32r=None):
    if _use_f32r is None:
        _use_f32r = os.environ.get("BD_F32R", "1") == "1"
    trace = _trace or os.environ.get("BD_TRACE", "0") == "1"
    from concourse.bass_utils import run_bass_kernel_spmd

    nc = _build_program(use_f32r=_use_f32r)
    in_maps = _host_prep(q_alpha, k, h_norm, gamma_vec, causal_mask)
    kwargs = {}
    if trace:
        _ensure_ntff_hook()
        import concourse.bass_utils as bu
        bu.upload_artifacts = lambda tmpdir: tmpdir  # no bucket in container
        kwargs = dict(trace=True, tmpdir=os.environ.get("BD_TRACE_DIR") or None)
    res = run_bass_kernel_spmd(nc, in_maps, list(range(B)), **kwargs)
    _last["exec_time_ns"] = res.exec_time_ns
    out = np.empty((B, W, D), np.float32)
    for b in range(B):
        out[b] = res.results[b]["otT"].T
    return out
